# revision 2
# baseline (speedup 1.0000x reference)
"""Trainium2 Bass kernel for nn_ContrastiveLoss (wav2vec2-style) — v2.

Shapes (hardcoded): B=8, C=256, T=1024, M=512 masked positions, K=100
negatives. Sharding: pure data parallel — batch row b -> NeuronCore b.

v2 design: ONE fp8 stream of the negatives (13.4 MB/core, vs 28.6 MB in v1)
in c-major DoubleRow-interleaved layout. The PE computes BOTH the dot
products and the sum-of-squares:

- dots^T[k, m]  = matmul(lhsT=negT[m] [128, 2, 128] fp8, rhs=ctx[m] [128,2,1],
                  perf_mode=DoubleRow)  -- contracts all 256 channels at once.
- ssq^T[k, m]   = matmul(lhsT=sq[m], rhs=ones [128,2,1], DoubleRow) where
                  sq = negT^2 elementwise.

The elementwise squares (the v1 bottleneck: 85us on ScalarE alone) are split
across ScalarE / VectorE / GpSimd in one big op each per streamed tile,
fp8 -> fp8, proportional to each engine's rate. Both PE outputs land [k, m]
in PSUM and share the copy + PE-transpose epilogue into [m, k].

Streams (host-prepped):
- negt [HG=8, 128, 2, 6528] fp8 (13.4 MB): negt[hg, c', i, ml*K+k] =
  neg[hg*64+ml, k, i*128+c']; cols 6400..6528 are zero pad so every
  128-col lhsT slice stays in bounds.
- ctxt [128, 2, M] fp8: ctx chunks for the DoubleRow moving operand.
- ctxg/posg [M, C] bf16: pos-similarity + ctx/pos norms (exact-ish).
"""

import numpy as np

TEMP = 0.1
EPS = 1e-8
B, C, T = 8, 256, 1024
M = 512  # masked positions per batch row
K = 100  # negatives per masked position
P = 128  # partitions
G = M // P  # m-groups per core (4)
HG = 8  # half-groups (64 m's each)
MH = M // HG  # 64
NCOL = MH * K + P  # 6528: streamed tile cols incl. 128-col pad
# elementwise-square split (cols of each [128, 2, NCOL] tile)
# rates: ACT 0.833 ns/col, DVE ~1.04 (fp8 1x), Pool ~1.98
ACT_SQ = 2780
DVE_SQ = 2320
POOL_SQ = NCOL - ACT_SQ - DVE_SQ  # 1428

_NC = None


def _build_nc():
    import concourse.bacc as bacc
    import concourse.tile as tile
    from concourse import masks, mybir

    f32 = mybir.dt.float32
    bf16 = mybir.dt.bfloat16
    fp8 = mybir.dt.float8e4
    Alu = mybir.AluOpType
    Act = mybir.ActivationFunctionType
    DR = mybir.MatmulPerfMode.DoubleRow

    nc = bacc.Bacc(trn_type="TRN2")
    negt = nc.dram_tensor("negt", [HG, P, 2, NCOL], fp8, kind="ExternalInput")
    ctxt = nc.dram_tensor("ctxt", [P, 2, M], fp8, kind="ExternalInput")
    ctxg = nc.dram_tensor("ctxg", [M, C], bf16, kind="ExternalInput")
    posg = nc.dram_tensor("posg", [M, C], bf16, kind="ExternalInput")
    rowloss = nc.dram_tensor("rowloss", [P, G], f32, kind="ExternalOutput")

    from contextlib import ExitStack

    with tile.TileContext(nc) as tc, ExitStack() as es:
        pool_specs = dict(single=1, ntp=3, sqp=3, grp=2, pg=G, scrp=2, scp=2)
        pools = {
            n: es.enter_context(tc.tile_pool(name=n, bufs=b))
            for n, b in pool_specs.items()
        }
        single, ntp, sqp, grp, pg, scrp, scp = (
            pools[n] for n in ("single", "ntp", "sqp", "grp", "pg", "scrp", "scp")
        )
        accp = es.enter_context(tc.psum_pool(name="acc", bufs=2))
        tpp = es.enter_context(tc.psum_pool(name="tp", bufs=2))

        identity = single.tile([P, P], f32)
        masks.make_identity(nc, identity[:])
        ones_mov = single.tile([P, 2, 1], fp8, name="ones")
        nc.gpsimd.memset(ones_mov[:], 1.0)
        dummy = single.tile([P, C], bf16)

        ctxt_s = single.tile([P, 2, M], fp8, name="ctxts")
        nc.sync.dma_start(out=ctxt_s[:], in_=ctxt[:, :, :])

        css_a = single.tile([P, G], f32)
        pss_a = single.tile([P, G], f32)
        cpd_a = single.tile([P, G], f32)
        crn_a = single.tile([P, G], f32)
        prn_a = single.tile([P, G], f32)
        se_a = single.tile([P, G], f32)
        lnse_a = single.tile([P, G], f32)
        nss_a = single.tile([P, G * K], f32)
        nrn_a = single.tile([P, G * K], f32)
        out_t = single.tile([P, G], f32)

        gt = {}
        for g in range(G):
            gt[g] = dict(
                logits=pg.tile([P, K + 1], f32, tag="logits", name=f"logits{g}"),
                dots=pg.tile([P, P], f32, tag="dots", name=f"dots{g}"),
            )

        # ctx/pos norms + ctx.pos dot: independent of the negatives stream
        for g in range(G):
            m0 = g * P
            ctx_t = grp.tile([P, C], bf16, tag="ctx")
            pos_t = grp.tile([P, C], bf16, tag="pos")
            nc.sync.dma_start(out=ctx_t[:], in_=ctxg[m0 : m0 + P, :])
            nc.sync.dma_start(out=pos_t[:], in_=posg[m0 : m0 + P, :])
            nc.vector.scalar_tensor_tensor(
                out=dummy[:], in0=ctx_t[:], scalar=1.0, in1=ctx_t[:],
                op0=Alu.mult, op1=Alu.mult, accum_out=css_a[:, g : g + 1],
            )
            nc.vector.scalar_tensor_tensor(
                out=dummy[:], in0=pos_t[:], scalar=1.0, in1=pos_t[:],
                op0=Alu.mult, op1=Alu.mult, accum_out=pss_a[:, g : g + 1],
            )
            nc.vector.scalar_tensor_tensor(
                out=dummy[:], in0=ctx_t[:], scalar=1.0, in1=pos_t[:],
                op0=Alu.mult, op1=Alu.mult, accum_out=cpd_a[:, g : g + 1],
            )

        # ---- main stream: per half-group square + 128 DoubleRow matmuls ----
        acc = {}
        for hg in range(HG):
            g, half = hg // 2, hg % 2
            nt = ntp.tile([P, 2, NCOL], fp8, tag="nt")
            nc.sync.dma_start(out=nt[:], in_=negt[hg])
            sq = sqp.tile([P, 2, NCOL], fp8, tag="sq")
            with nc.allow_low_precision(reason="fp8 squares feed fp8 matmul"):
                nc.scalar.activation(
                    out=sq[:, :, 0:ACT_SQ], in_=nt[:, :, 0:ACT_SQ], func=Act.Square
                )
                nc.vector.tensor_tensor(
                    out=sq[:, :, ACT_SQ : ACT_SQ + DVE_SQ],
                    in0=nt[:, :, ACT_SQ : ACT_SQ + DVE_SQ],
                    in1=nt[:, :, ACT_SQ : ACT_SQ + DVE_SQ],
                    op=Alu.mult,
                )
                nc.gpsimd.tensor_tensor(
                    out=sq[:, :, ACT_SQ + DVE_SQ : NCOL],
                    in0=nt[:, :, ACT_SQ + DVE_SQ : NCOL],
                    in1=nt[:, :, ACT_SQ + DVE_SQ : NCOL],
                    op=Alu.mult,
                )
            if half == 0:
                acc[g] = accp.tile([P, 2 * P], f32, tag="acc", name=f"acc{g}")
            for ml in range(MH):
                mg = half * MH + ml
                m = hg * MH + ml
                c0 = ml * K
                nc.tensor.matmul(
                    out=acc[g][:, mg : mg + 1],
                    lhsT=nt[:, :, c0 : c0 + P],
                    rhs=ctxt_s[:, :, m : m + 1],
                    start=True, stop=True, perf_mode=DR,
                )
                nc.tensor.matmul(
                    out=acc[g][:, P + mg : P + mg + 1],
                    lhsT=sq[:, :, c0 : c0 + P],
                    rhs=ones_mov[:],
                    start=True, stop=True, perf_mode=DR,
                )
            if half == 1:
                accsb = scp.tile([P, 2 * P], f32, tag="accsb")
                nc.vector.tensor_copy(accsb[:], acc[g][:])
                tp = tpp.tile([P, 2 * P], f32, tag="tp")
                nc.tensor.transpose(tp[:, 0:P], accsb[:, 0:P], identity[:])
                nc.tensor.transpose(tp[:, P : 2 * P], accsb[:, P : 2 * P], identity[:])
                nc.vector.tensor_copy(gt[g]["dots"][:], tp[:, 0:P])
                nc.scalar.sqrt(nss_a[:, g * K : (g + 1) * K], tp[:, P : P + K])

        # ---- epilogue ----
        nc.scalar.sqrt(css_a[:], css_a[:])
        nc.scalar.sqrt(pss_a[:], pss_a[:])
        nc.vector.tensor_scalar_max(css_a[:], css_a[:], EPS)
        nc.vector.tensor_scalar_max(pss_a[:], pss_a[:], EPS)
        nc.vector.reciprocal(crn_a[:], css_a[:])
        nc.vector.reciprocal(prn_a[:], pss_a[:])
        nc.vector.tensor_scalar_max(nss_a[:], nss_a[:], EPS)
        nc.vector.reciprocal(nrn_a[:], nss_a[:])
        for g in range(G):
            d = gt[g]
            nc.vector.scalar_tensor_tensor(
                out=d["logits"][:, 0:1], in0=cpd_a[:, g : g + 1],
                scalar=crn_a[:, g : g + 1], in1=prn_a[:, g : g + 1],
                op0=Alu.mult, op1=Alu.mult,
            )
            nc.vector.scalar_tensor_tensor(
                out=d["logits"][:, 1 : K + 1], in0=d["dots"][:, 0:K],
                scalar=crn_a[:, g : g + 1], in1=nrn_a[:, g * K : (g + 1) * K],
                op0=Alu.mult, op1=Alu.mult,
            )
        # |logits| <= 1 so exp(logit/TEMP) <= e^10 — no max-shift needed
        for g in range(G):
            d = gt[g]
            esc = scrp.tile([P, K + 1], f32, tag="esc")
            nc.scalar.activation(
                out=esc[:], in_=d["logits"][:], func=Act.Exp,
                scale=1.0 / TEMP, accum_out=se_a[:, g : g + 1],
            )
        nc.scalar.activation(out=lnse_a[:], in_=se_a[:], func=Act.Ln)
        for g in range(G):
            nc.vector.scalar_tensor_tensor(
                out=out_t[:, g : g + 1], in0=gt[g]["logits"][:, 0:1],
                scalar=-1.0 / TEMP, in1=lnse_a[:, g : g + 1],
                op0=Alu.mult, op1=Alu.add,
            )
        nc.sync.dma_start(out=rowloss[:], in_=out_t[:])
    nc.finalize()
    return nc


def _get_nc():
    global _NC
    if _NC is None:
        _NC = _build_nc()
    return _NC


def make_in_maps(context, positive, negatives, mask_indices):
    import ml_dtypes

    bf = ml_dtypes.bfloat16
    f8 = ml_dtypes.float8_e4m3
    context = np.asarray(context, dtype=np.float32)
    positive = np.asarray(positive, dtype=np.float32)
    negatives = np.asarray(negatives, dtype=np.float32)
    mask = np.asarray(mask_indices).astype(bool)
    in_maps = []
    for b in range(B):
        idx = np.flatnonzero(mask[b])
        assert idx.size == M, f"row {b}: expected {M} masked, got {idx.size}"
        ctx_m = np.ascontiguousarray(context[b].T[idx])  # [M, C] f32
        pos_m = np.ascontiguousarray(positive[b].T[idx])  # [M, C] f32
        neg = negatives[b]  # [M, K, C] f32
        # negT [2, 128, M, K]: c split into (i=c//128, c'=c%128)
        negT = neg.transpose(2, 0, 1).reshape(2, P, M, K).astype(f8)
        nt = np.zeros((HG, P, 2, NCOL), dtype=f8)
        blk = (
            negT.reshape(2, P, HG, MH * K)
            .transpose(2, 1, 0, 3)  # [HG, P, 2, MH*K]
        )
        nt[:, :, :, : MH * K] = blk
        ctxT = ctx_m.T.astype(f8)  # [C, M]
        ctxt = np.ascontiguousarray(
            ctxT.reshape(2, P, M).transpose(1, 0, 2)
        )  # [P, 2, M]
        in_maps.append(
            {
                "negt": nt,
                "ctxt": ctxt,
                "ctxg": ctx_m.astype(bf),
                "posg": pos_m.astype(bf),
            }
        )
    return in_maps


def kernel(context, positive, negatives, mask_indices, num_masked):
    from concourse.bass_utils import run_bass_kernel_spmd

    nm = int(np.asarray(num_masked))
    assert nm == M, f"kernel hardcodes num_masked={M}, got {nm}"
    assert np.asarray(context).shape == (B, C, T)
    assert np.asarray(negatives).shape == (B, M, K, C)

    in_maps = make_in_maps(context, positive, negatives, mask_indices)
    res = run_bass_kernel_spmd(_get_nc(), in_maps, core_ids=list(range(B)))
    total = np.float64(0.0)
    for r in res.results:
        total += r["rowloss"].astype(np.float64).sum()
    return np.float32(total / (B * M))


# revision 3
# speedup vs baseline: 1.3804x; 1.3804x over previous
"""Trainium2 Bass kernel for nn_ContrastiveLoss (wav2vec2-style) — v2.

Shapes (hardcoded): B=8, C=256, T=1024, M=512 masked positions, K=100
negatives. Sharding: pure data parallel — batch row b -> NeuronCore b.

v2 design: ONE fp8 stream of the negatives (13.4 MB/core, vs 28.6 MB in v1)
in c-major DoubleRow-interleaved layout. The PE computes BOTH the dot
products and the sum-of-squares:

- dots^T[k, m]  = matmul(lhsT=negT[m] [128, 2, 128] fp8, rhs=ctx[m] [128,2,1],
                  perf_mode=DoubleRow)  -- contracts all 256 channels at once.
- ssq^T[k, m]   = matmul(lhsT=sq[m], rhs=ones [128,2,1], DoubleRow) where
                  sq = negT^2 elementwise.

The elementwise squares (the v1 bottleneck: 85us on ScalarE alone) are split
across ScalarE / VectorE / GpSimd in one big op each per streamed tile,
fp8 -> fp8, proportional to each engine's rate. Both PE outputs land [k, m]
in PSUM and share the copy + PE-transpose epilogue into [m, k].

Streams (host-prepped):
- negt [HG=8, 128, 2, 6528] fp8 (13.4 MB): negt[hg, c', i, ml*K+k] =
  neg[hg*64+ml, k, i*128+c']; cols 6400..6528 are zero pad so every
  128-col lhsT slice stays in bounds.
- ctxt [128, 2, M] fp8: ctx chunks for the DoubleRow moving operand.
- ctxg/posg [M, C] bf16: pos-similarity + ctx/pos norms (exact-ish).
"""

import numpy as np

TEMP = 0.1
EPS = 1e-8
B, C, T = 8, 256, 1024
M = 512  # masked positions per batch row
K = 100  # negatives per masked position
P = 128  # partitions
G = M // P  # m-groups per core (4)
HG = 8  # half-groups (64 m's each)
MH = M // HG  # 64
NCOL = MH * K + P  # 6528: streamed tile cols incl. 128-col pad
# elementwise-square split (cols of each [128, 2, NCOL] tile)
# rates: ACT 0.833 ns/col, DVE ~1.04 (fp8 1x), Pool ~1.98
ACT_SQ = 2993
DVE_SQ = 2141
POOL_SQ = NCOL - ACT_SQ - DVE_SQ  # 1394

_NC = None


def _build_nc():
    import concourse.bacc as bacc
    import concourse.tile as tile
    from concourse import masks, mybir

    f32 = mybir.dt.float32
    bf16 = mybir.dt.bfloat16
    fp8 = mybir.dt.float8e4
    Alu = mybir.AluOpType
    Act = mybir.ActivationFunctionType
    DR = mybir.MatmulPerfMode.DoubleRow

    nc = bacc.Bacc(trn_type="TRN2")
    negt = nc.dram_tensor("negt", [HG, P, 2, NCOL], fp8, kind="ExternalInput")
    ctxt = nc.dram_tensor("ctxt", [P, 2, M], fp8, kind="ExternalInput")
    ctxg = nc.dram_tensor("ctxg", [M, C], bf16, kind="ExternalInput")
    posg = nc.dram_tensor("posg", [M, C], bf16, kind="ExternalInput")
    rowloss = nc.dram_tensor("rowloss", [P, G], f32, kind="ExternalOutput")

    from contextlib import ExitStack

    with tile.TileContext(nc) as tc, ExitStack() as es:
        pool_specs = dict(single=1, ntp=3, sqp=3, grp=2, pg=G, scrp=2, scp=2)
        pools = {
            n: es.enter_context(tc.tile_pool(name=n, bufs=b))
            for n, b in pool_specs.items()
        }
        single, ntp, sqp, grp, pg, scrp, scp = (
            pools[n] for n in ("single", "ntp", "sqp", "grp", "pg", "scrp", "scp")
        )
        accp = es.enter_context(tc.psum_pool(name="acc", bufs=2))
        tpp = es.enter_context(tc.psum_pool(name="tp", bufs=G))

        identity = single.tile([P, P], f32)
        masks.make_identity(nc, identity[:])
        ones_mov = single.tile([P, 1], fp8, name="ones")
        nc.gpsimd.memset(ones_mov[:], 1.0)
        dummy = single.tile([P, C], bf16)

        ctxt_s = single.tile([P, 2, M], fp8, name="ctxts")
        nc.sync.dma_start(out=ctxt_s[:], in_=ctxt[:, :, :])

        css_a = single.tile([P, G], f32)
        pss_a = single.tile([P, G], f32)
        cpd_a = single.tile([P, G], f32)
        crn_a = single.tile([P, G], f32)
        prn_a = single.tile([P, G], f32)
        se_a = single.tile([P, G], f32)
        lnse_a = single.tile([P, G], f32)
        nss_a = single.tile([P, G * K], f32)
        nrn_a = single.tile([P, G * K], f32)
        out_t = single.tile([P, G], f32)

        gt = {}
        for g in range(G):
            gt[g] = dict(
                logits=pg.tile([P, K + 1], f32, tag="logits", name=f"logits{g}"),
            )

        # ctx/pos norms + ctx.pos dot: independent of the negatives stream
        for g in range(G):
            m0 = g * P
            ctx_t = grp.tile([P, C], bf16, tag="ctx")
            pos_t = grp.tile([P, C], bf16, tag="pos")
            nc.sync.dma_start(out=ctx_t[:], in_=ctxg[m0 : m0 + P, :])
            nc.sync.dma_start(out=pos_t[:], in_=posg[m0 : m0 + P, :])
            nc.vector.scalar_tensor_tensor(
                out=dummy[:], in0=ctx_t[:], scalar=1.0, in1=ctx_t[:],
                op0=Alu.mult, op1=Alu.mult, accum_out=css_a[:, g : g + 1],
            )
            nc.vector.scalar_tensor_tensor(
                out=dummy[:], in0=pos_t[:], scalar=1.0, in1=pos_t[:],
                op0=Alu.mult, op1=Alu.mult, accum_out=pss_a[:, g : g + 1],
            )
            nc.vector.scalar_tensor_tensor(
                out=dummy[:], in0=ctx_t[:], scalar=1.0, in1=pos_t[:],
                op0=Alu.mult, op1=Alu.mult, accum_out=cpd_a[:, g : g + 1],
            )

        # ---- main stream: per half-group square + 128 DoubleRow matmuls ----
        acc = {}
        for hg in range(HG):
            g, half = hg // 2, hg % 2
            nt = ntp.tile([P, 2, NCOL], fp8, tag="nt")
            nc.sync.dma_start(out=nt[:], in_=negt[hg])
            sq = sqp.tile([P, 2, NCOL], fp8, tag="sq")
            with nc.allow_low_precision(reason="fp8 squares feed fp8 matmul"):
                nc.scalar.activation(
                    out=sq[:, :, 0:ACT_SQ], in_=nt[:, :, 0:ACT_SQ], func=Act.Square
                )
                nc.vector.tensor_tensor(
                    out=sq[:, :, ACT_SQ : ACT_SQ + DVE_SQ],
                    in0=nt[:, :, ACT_SQ : ACT_SQ + DVE_SQ],
                    in1=nt[:, :, ACT_SQ : ACT_SQ + DVE_SQ],
                    op=Alu.mult,
                )
                nc.gpsimd.tensor_tensor(
                    out=sq[:, :, ACT_SQ + DVE_SQ : NCOL],
                    in0=nt[:, :, ACT_SQ + DVE_SQ : NCOL],
                    in1=nt[:, :, ACT_SQ + DVE_SQ : NCOL],
                    op=Alu.mult,
                )
            if half == 0:
                acc[g] = accp.tile([P, 2 * P], f32, tag="acc", name=f"acc{g}")
            for ml in range(MH):
                mg = half * MH + ml
                m = hg * MH + ml
                c0 = ml * K
                for ch in range(2):
                    nc.tensor.matmul(
                        out=acc[g][:, mg : mg + 1],
                        lhsT=nt[:, ch, c0 : c0 + P],
                        rhs=ctxt_s[:, ch, m : m + 1],
                        start=(ch == 0), stop=(ch == 1),
                    )
                for ch in range(2):
                    nc.tensor.matmul(
                        out=acc[g][:, P + mg : P + mg + 1],
                        lhsT=sq[:, ch, c0 : c0 + P],
                        rhs=ones_mov[:],
                        start=(ch == 0), stop=(ch == 1),
                    )
            if half == 1:
                accsb = scp.tile([P, 2 * P], f32, tag="accsb")
                nc.vector.tensor_copy(accsb[:], acc[g][:])
                tp = tpp.tile([P, 2 * P], f32, tag="tp", name=f"tp{g}")
                nc.tensor.transpose(tp[:, 0:P], accsb[:, 0:P], identity[:])
                nc.tensor.transpose(tp[:, P : 2 * P], accsb[:, P : 2 * P], identity[:])
                gt[g]["tp"] = tp

        # ---- epilogue ----
        nc.scalar.sqrt(css_a[:], css_a[:])
        nc.scalar.sqrt(pss_a[:], pss_a[:])
        nc.vector.tensor_scalar_max(css_a[:], css_a[:], EPS)
        nc.vector.tensor_scalar_max(pss_a[:], pss_a[:], EPS)
        nc.vector.reciprocal(crn_a[:], css_a[:])
        nc.vector.reciprocal(prn_a[:], pss_a[:])
        for g in range(G):
            nc.scalar.sqrt(
                nss_a[:, g * K : (g + 1) * K], gt[g]["tp"][:, P : P + K]
            )
        nc.vector.tensor_scalar_max(nss_a[:], nss_a[:], EPS)
        nc.vector.reciprocal(nrn_a[:], nss_a[:])
        for g in range(G):
            d = gt[g]
            nc.vector.scalar_tensor_tensor(
                out=d["logits"][:, 0:1], in0=cpd_a[:, g : g + 1],
                scalar=crn_a[:, g : g + 1], in1=prn_a[:, g : g + 1],
                op0=Alu.mult, op1=Alu.mult,
            )
            nc.vector.scalar_tensor_tensor(
                out=d["logits"][:, 1 : K + 1], in0=d["tp"][:, 0:K],
                scalar=crn_a[:, g : g + 1], in1=nrn_a[:, g * K : (g + 1) * K],
                op0=Alu.mult, op1=Alu.mult,
            )
        # |logits| <= 1 so exp(logit/TEMP) <= e^10 — no max-shift needed
        for g in range(G):
            d = gt[g]
            esc = scrp.tile([P, K + 1], f32, tag="esc")
            nc.scalar.activation(
                out=esc[:], in_=d["logits"][:], func=Act.Exp,
                scale=1.0 / TEMP, accum_out=se_a[:, g : g + 1],
            )
        nc.scalar.activation(out=lnse_a[:], in_=se_a[:], func=Act.Ln)
        for g in range(G):
            nc.vector.scalar_tensor_tensor(
                out=out_t[:, g : g + 1], in0=gt[g]["logits"][:, 0:1],
                scalar=-1.0 / TEMP, in1=lnse_a[:, g : g + 1],
                op0=Alu.mult, op1=Alu.add,
            )
        nc.sync.dma_start(out=rowloss[:], in_=out_t[:])
    nc.finalize()
    return nc


def _get_nc():
    global _NC
    if _NC is None:
        _NC = _build_nc()
    return _NC


def make_in_maps(context, positive, negatives, mask_indices):
    import ml_dtypes

    bf = ml_dtypes.bfloat16
    f8 = ml_dtypes.float8_e4m3
    context = np.asarray(context, dtype=np.float32)
    positive = np.asarray(positive, dtype=np.float32)
    negatives = np.asarray(negatives, dtype=np.float32)
    mask = np.asarray(mask_indices).astype(bool)
    in_maps = []
    for b in range(B):
        idx = np.flatnonzero(mask[b])
        assert idx.size == M, f"row {b}: expected {M} masked, got {idx.size}"
        ctx_m = np.ascontiguousarray(context[b].T[idx])  # [M, C] f32
        pos_m = np.ascontiguousarray(positive[b].T[idx])  # [M, C] f32
        neg = negatives[b]  # [M, K, C] f32
        # negT [2, 128, M, K]: c split into (i=c//128, c'=c%128)
        negT = neg.transpose(2, 0, 1).reshape(2, P, M, K).astype(f8)
        nt = np.zeros((HG, P, 2, NCOL), dtype=f8)
        blk = (
            negT.reshape(2, P, HG, MH * K)
            .transpose(2, 1, 0, 3)  # [HG, P, 2, MH*K]
        )
        nt[:, :, :, : MH * K] = blk
        ctxT = ctx_m.T.astype(f8)  # [C, M]
        ctxt = np.ascontiguousarray(
            ctxT.reshape(2, P, M).transpose(1, 0, 2)
        )  # [P, 2, M]
        in_maps.append(
            {
                "negt": nt,
                "ctxt": ctxt,
                "ctxg": ctx_m.astype(bf),
                "posg": pos_m.astype(bf),
            }
        )
    return in_maps


def kernel(context, positive, negatives, mask_indices, num_masked):
    from concourse.bass_utils import run_bass_kernel_spmd

    nm = int(np.asarray(num_masked))
    assert nm == M, f"kernel hardcodes num_masked={M}, got {nm}"
    assert np.asarray(context).shape == (B, C, T)
    assert np.asarray(negatives).shape == (B, M, K, C)

    in_maps = make_in_maps(context, positive, negatives, mask_indices)
    res = run_bass_kernel_spmd(_get_nc(), in_maps, core_ids=list(range(B)))
    total = np.float64(0.0)
    for r in res.results:
        total += r["rowloss"].astype(np.float64).sum()
    return np.float32(total / (B * M))
